# revision 42
# baseline (speedup 1.0000x reference)
"""Trainium2 Bass kernel for nn_CFNOTF_80066780332643.

The reference computes, per 16x16 patch p (flattened to 256 elems):
    y = Re(ifft(fft(p) @ (Wr+iWi) + (br-bi)+i(br+bi)))          [16 ch]
followed by a 3x3 depthwise conv (SAME) on the 256x256 patch grid,
inference BatchNorm, and nearest-resize 256->128 (which picks odd
rows/cols).  The fft->dense->ifft chain is linear in p, so it collapses
to a single real matmul  y = p @ M + d  with
    M = Re(E @ (Wr+iWi) @ G),  d = Re(c @ G)
(E = forward DFT matrix, G = inverse DFT matrix).  BN folds into a
per-channel scale/bias after the conv.

Per-core layout (8 cores, data-parallel over b*patch-rows): core c gets
image c//4, patch rows (c%4)*64 .. +64, plus one halo patch row below
(zeros + mask at the image bottom).  Each 128-pixel-row group is one
[128, 4096] SBUF tile: partitions = (hi 8, ki 16), free = (kj 16,
par 2, s 128) with wi = 2s+par -- the host pre-permutes columns so
that (a) each of the 16 stage-1 matmuls streams a CONTIGUOUS 256-elem
moving operand (strided rhs pays a fresh 16B-SBUF-line penalty per
element, 4x slower), and (b) stage-1 PSUM comes out parity-split so
the stage-2 conv taps (wi = 2s, 2s+1, 2s+2) are contiguous 128-col
slices of y.

x streams in float8 e3m4 (pre-scaled 2x on the host so N(0,1) sits in
the normal range; the 1/2 folds into the fp16 stationaries) -- mixed
fp8-moving x fp16-stationary matmuls are supported and halve the input
DMA, which makes the kernel PE-column-rate-bound (1 col/cycle).

Stage-1 y[(hi,oc), (par,s)] accumulates over 16 matmuls per group in
PSUM (block-diag [128,128] stationaries, expanded on-chip by DVE from
a compact [128,512] DMA -- the DVE exits the framework preamble before
any DMA completion sem can fire, so the expansion is off the critical
path), evacuated with +d bias.  The halo patch row ships host-
transposed ([128 patch-elem partitions, 512 cols]) so it contracts
128-deep in 2 matmuls (512 PE columns) instead of 16-deep in 16
(4096 columns), and runs right after group 0.  Stage-2
conv+BN+odd-subsample is 12 matmuls (3 dw taps x
{in-group, halo-row} x 2 halves, halo stationaries zero-padded to 128
rows so LDWEIGHTS stays pipelined); half 0 is interleaved mid stage-1
once groups 0-4 are evacuated, half 1 writes PSUM partitions 64..128
via the matmul tile_position col offset.  The [64, 512] output stores
(one per BN half, the first overlapping conv half 1) are made race-safe
by a trailing full-width guard DMA on the same HWDGE ring.
"""

import os
import sys

import ml_dtypes
import numpy as np

for _p in ("/opt/trn_rl_repo", "/root/.axon_site/_ro/trn_rl_repo"):
    if os.path.isdir(_p) and _p not in sys.path:
        sys.path.append(_p)

K = 16
D = 16
EPS = 1e-5
B, H, W = 2, 4096, 4096
HK, WK = H // K, W // K          # 256, 256
N_CORES = 8
GROUPS = 8                       # 8 full groups of 8 patch rows
GROUP_PIX = 128                  # pixel rows per full group
SLAB_ROWS = GROUPS * GROUP_PIX       # 1024 rows; halo ships transposed in xinh
YCOLS = 264                      # per-group y slot: 2 x (128 wi + 1 pad + 3 pad)
YHALF = 132
OUT_R = 4                        # output rows per group (odd rows 1,3,5,7)

MM_DTYPE = "float16"             # matmul dtype for both stages

# fp16 weights tile column layout (cstw carries the compact stage-1 data)
COL_LM = 0                       # [128, 192] conv main stationaries
COL_LH = 192                     # [128, 192] conv halo stationaries (rows 16+ zero)
CST16_W = 384
# f32 per-partition vectors tile columns
CV_DV = 0                        # [128] d bias
CV_MK = 1                        # [16] halo mask
CV_SV = 2                        # [128] BN scale
CV_BV = 3                        # [128] BN bias
CSTV_W = 4

LAST_RESULT = None               # BassKernelResults of the last run

# host-side column permutation: new col kj*256 + par*128 + s holds old
# pixel col wi*16 + kj with wi = 2s+par
_kj, _par, _s = np.meshgrid(np.arange(K), np.arange(2), np.arange(128),
                            indexing="ij")
_PERM = ((2 * _s + _par) * K + _kj).reshape(-1)


def _build_consts(Wr, br, Wi, bi, dw_kernel, dw_bias, gamma, beta,
                  moving_mean, moving_var):
    """Host-side: collapse fft/dense/ifft to M3 [ki,kj,oc], d [oc]; BN scale/bias."""
    Wr = Wr.astype(np.float64)
    Wi = Wi.astype(np.float64)
    fin = K * K
    m = np.arange(fin)
    E = np.exp(-2j * np.pi * np.outer(m, m) / fin)
    n = np.arange(D)
    G = np.exp(2j * np.pi * np.outer(n, n) / D) / D
    M = np.real(E @ (Wr + 1j * Wi) @ G)                  # (256, 16)
    d = np.real(((br - bi) + 1j * (br + bi)).astype(np.complex128) @ G)  # (16,)
    M3 = M.reshape(K, K, D)                              # [ki, kj, oc]

    a_vec = (gamma.astype(np.float64)
             / np.sqrt(moving_var.astype(np.float64) + EPS))
    bias_vec = a_vec * (dw_bias.astype(np.float64)
                        - moving_mean.astype(np.float64)) + beta.astype(np.float64)

    # conv stationaries: main [3(dw), 128, 64] with dh taps as bands,
    # halo [3(dw), 16, 64] for the row-8 (next group row 0) tap.
    dwk = dw_kernel[..., 0].astype(np.float32)           # (3, 3, 16) [dh, dw, oc]
    Lmain = np.zeros((3, 128, 64), np.float32)
    # halo stationary padded to 128 rows: rows 16.. are zero, so the
    # matmul can stream the full 128-partition y tile (uniform LDWEIGHTS
    # shape keeps the background weight-load pipelined)
    Lhalo = np.zeros((3, 128, 64), np.float32)
    for dwi in range(3):
        for r in range(OUT_R):
            for dhi in range(3):
                hi = 2 * r + dhi            # tap row = 2r+1 + (dhi-1)
                for oc in range(D):
                    if hi < 8:
                        Lmain[dwi, hi * 16 + oc, r * 16 + oc] = dwk[dhi, dwi, oc]
                    else:
                        Lhalo[dwi, oc, r * 16 + oc] = dwk[dhi, dwi, oc]

    dvec = np.tile(d.astype(np.float32), 8)              # [128]
    scalev = np.tile(a_vec.astype(np.float32), 2 * OUT_R)    # [128]
    bvec = np.tile(bias_vec.astype(np.float32), 2 * OUT_R)   # [128]
    return M3, Lmain, Lhalo, dvec, scalev, bvec


def _build_nc():
    import concourse.bass as bass
    from concourse import mybir

    mmdt = getattr(mybir.dt, MM_DTYPE)
    xdt = mybir.dt.float8e3
    f32 = mybir.dt.float32

    nc = bass.Bass()
    xin = nc.declare_dram_parameter("xin", [SLAB_ROWS, W], xdt, isOutput=False)
    cst = nc.declare_dram_parameter("cst", [128, CST16_W], mmdt, isOutput=False)
    cstw = nc.declare_dram_parameter("cstw", [128, 544], mmdt, isOutput=False)
    xinh = nc.declare_dram_parameter("xinh", [128, 2 * WK], xdt, isOutput=False)
    cstv = nc.declare_dram_parameter("cstv", [128, CSTV_W], f32, isOutput=False)
    out_d = nc.declare_dram_parameter("out", [128, OUT_R * GROUP_PIX], f32,
                                      isOutput=True)

    ct = nc.alloc_sbuf_tensor("ct", [128, CST16_W], mmdt)
    ctw = nc.alloc_sbuf_tensor("ctw", [128, 544], mmdt)
    xht = nc.alloc_sbuf_tensor("xht", [128, 2 * WK], xdt)
    ctv = nc.alloc_sbuf_tensor("ctv", [128, CSTV_W], f32)
    w2t = nc.alloc_sbuf_tensor("w2t", [128, K * 128], mmdt)
    xgs = [nc.alloc_sbuf_tensor(f"xg{g}", [128, W], xdt)
           for g in range(GROUPS)]
    guard = nc.alloc_sbuf_tensor("guard", [128, 8], xdt)
    y_all = nc.alloc_sbuf_tensor("y_all", [128, 9 * YCOLS], mmdt)
    out_sb = nc.alloc_sbuf_tensor("out_sb", [128, OUT_R * GROUP_PIX], f32)
    # yp: two half-bank accumulation regions in two separate banks (bank
    # exclusivity vs the DVE evacuation read -- PE-W + DVE-R on one bank
    # is a hardware fault).  halop: own bank so the halo matmuls (issued
    # first) never conflict with group evacuations.  convp: half h at
    # partitions h*64 and bank h, so conv-half-1 writes never share a
    # bank with the BN read of half 0.
    yp = nc.alloc_psum_tensor("yp", [128, 1024], f32)
    halop = nc.alloc_psum_tensor("halop", [K, WK], f32)
    convp = nc.alloc_psum_tensor("convp", [128, 1024], f32)

    ctap = ct[:]
    cvap = ctv[:]
    dvt = cvap[:, CV_DV:CV_DV + 1]
    mkt = cvap[0:16, CV_MK:CV_MK + 1]
    svt = cvap[:, CV_SV:CV_SV + 1]
    bvt = cvap[:, CV_BV:CV_BV + 1]
    yav = y_all[:]
    yv3 = yav.rearrange("p (g c) -> p g c", c=YCOLS)
    yv4 = yav.rearrange("p (g t s) -> p g t s", g=9, s=YHALF)
    w2v = w2t[:].rearrange("p (k c) -> p k c", c=128)
    # conv tap column slices: y[2s] -> t0 s, y[2s+1] -> t1 s, y[2s+2] -> t0 s+1
    conv_slices = [slice(0, 128), slice(YHALF, YHALF + 128), slice(1, 129)]

    with (
        nc.Block() as block,
        nc.semaphore("s_cst") as s_cst,
        nc.semaphore("s_cstv") as s_cstv,
        nc.semaphore("s_cw") as s_cw,
        nc.semaphore("s_w2") as s_w2,
        nc.semaphore("s_w2b") as s_w2b,
        nc.semaphore("s_ms") as s_ms,
        nc.semaphore("s_pad") as s_pad,
        nc.semaphore("s_og") as s_og,
        nc.semaphore("s_hg") as s_hg,
        nc.semaphore("s_x0b") as s_x0b,
        nc.semaphore("s_x7b") as s_x7b,
        nc.semaphore("s_pe") as s_pe,
        nc.semaphore("s_evac") as s_evac,
        nc.semaphore("s_conv") as s_conv,
        nc.semaphore("s_out") as s_out,
        nc.semaphore("s_od") as s_od,
    ):
        x_sems = [nc.alloc_semaphore(name=f"s_x{g}") for g in range(GROUPS)]

        @block.sync
        def _(sync: bass.BassEngine):
            # everything on ONE HWDGE ring (the second, scalar/ACT ring is
            # starved whenever this one has a backlog), in criticality
            # order: the compact consts (the stage-1 stationary expansion
            # gates all matmuls), then group 0 split in two so its matmuls
            # start on the first half.
            sync.dma_start(
                out=xgs[0][:, 0:W // 4],
                in_=xin[0:GROUP_PIX, 0:W // 4],
            ).then_inc(x_sems[0], 16)
            sync.dma_start(out=ctw[:], in_=cstw[:]).then_inc(s_cw, 16)
            sync.dma_start(out=ctap, in_=cst[:]).then_inc(s_cst, 16)
            sync.dma_start(out=cvap, in_=cstv[:]).then_inc(s_cstv, 16)
            sync.dma_start(
                out=xgs[0][:, W // 4:W],
                in_=xin[0:GROUP_PIX, W // 4:W],
            ).then_inc(s_x0b, 16)
            # halo patch row, host-transposed to [128 (kj,ki), 2x256]:
            # full-width (sem-race-free), and it lets the halo contract
            # 128-deep in 2 matmuls (512 PE columns) instead of 16-deep in
            # 16 matmuls (4096 columns).
            sync.dma_start(out=xht[:], in_=xinh[:]).then_inc(s_hg, 16)
            for g in range(1, GROUPS):
                if g < GROUPS - 1:
                    sync.dma_start(
                        out=xgs[g][:],
                        in_=xin[g * GROUP_PIX:(g + 1) * GROUP_PIX, :],
                    ).then_inc(x_sems[g], 16)
                else:
                    # last group also split: tail shrinks by half a group
                    sync.dma_start(
                        out=xgs[g][:, 0:W // 2],
                        in_=xin[g * GROUP_PIX:(g + 1) * GROUP_PIX, 0:W // 2],
                    ).then_inc(x_sems[g], 16)
                    sync.dma_start(
                        out=xgs[g][:, W // 2:W],
                        in_=xin[g * GROUP_PIX:(g + 1) * GROUP_PIX, W // 2:W],
                    ).then_inc(s_x7b, 16)

        def conv_mms(tensor, half, g0, ng, c0):
            cvh = convp[half * 64:half * 64 + 64,
                        c0:c0 + ng * 128].rearrange("p (g s) -> p g s", g=ng)
            mm = None
            for i, (col, goff) in enumerate(
                    [(COL_LM, 0)] * 3 + [(COL_LH, 1)] * 3):
                dwi = i % 3
                mm = tensor.matmul(
                    cvh,
                    ctap[:, col + dwi * 64:col + dwi * 64 + 64],
                    yv3[:, g0 + goff:g0 + goff + ng, conv_slices[dwi]],
                    start=(i == 0), stop=(i == 5),
                )
            mm.then_inc(s_conv, 1)

        @block.tensor
        def _(tensor: bass.BassEngine):
            tensor.wait_ge(s_w2, 4)
            for g in range(GROUPS):
                tensor.wait_ge(x_sems[g], 16)
                if g >= 2:
                    # the target psum half-bank is free once evac(g-2)
                    # completed (s_evac incs: g0=1, halo=2, g1=3, ...)
                    tensor.wait_ge(s_evac, 1 if g == 2 else g)
                xv = xgs[g][:].rearrange("p (k w) -> p k w", w=WK)
                ypg = yp[:, (g % 2) * 512:(g % 2) * 512 + WK]
                for kj in range(K):
                    if g == 0 and kj == 4:
                        tensor.wait_ge(s_w2b, 4)
                        tensor.wait_ge(s_x0b, 16)
                    if g == GROUPS - 1 and kj == K // 2:
                        tensor.wait_ge(s_x7b, 16)
                    mm = tensor.matmul(
                        ypg,
                        w2v[:, kj, :],
                        xv[:, kj, :],
                        start=(kj == 0),
                        stop=(kj == K - 1),
                    )
                mm.then_inc(s_pe, 1)
                if g == 0:
                    # halo patch row right after group 0, transposed form:
                    # partitions = 128 patch elements (kj-half, ki), so two
                    # 128-deep matmuls of 256 cols produce y[oc, (t,s)]
                    tensor.wait_ge(s_hg, 16)
                    for b in range(2):
                        mm = tensor.matmul(
                            halop[:],
                            ctw[:, 512 + b * 16:512 + b * 16 + 16],
                            xht[:, b * WK:(b + 1) * WK],
                            start=(b == 0), stop=(b == 1),
                        )
                    mm.then_inc(s_pe, 1)
                if g == 5:
                    # conv half 0 fits in the DMA-paced idle window once
                    # groups 0-4 (its taps) are evacuated
                    tensor.wait_ge(s_cst, 16)
                    tensor.wait_ge(s_pad, 2)
                    tensor.wait_ge(s_evac, 6)
                    conv_mms(tensor, 0, 0, 4, 0)
            # conv half 1 split at a group boundary: the groups-4/5
            # sub-block only needs evacs through g6, so it runs while
            # g7's evacuation is still in flight
            tensor.wait_ge(s_evac, 8)
            conv_mms(tensor, 1, 4, 2, 512)
            tensor.wait_ge(s_evac, 9)
            conv_mms(tensor, 1, 6, 2, 768)

        @block.gpsimd
        def _(gpsimd: bass.BassEngine):
            # only y[256] (t=0, s=128) is ever read without being written
            gpsimd.memset(yv4[:, 0:GROUPS, 0, 128:129], 0.0).then_inc(s_pad, 1)
            gpsimd.memset(yv3[:, GROUPS, :], 0.0).then_inc(s_pad, 1)

        @block.vector
        def _(vector: bass.BassEngine):
            # expand the compact [128, 512] 32-diag data into the
            # block-diag [128, 16*128] stationary tile: memset + 4 copies
            # of 32-aligned diagonal blocks.  This engine exits the
            # framework preamble before any DMA data can land, and 0.13 MB
            # of compact data beats 0.5 MB of expanded DMA at the head of
            # the stream.
            vector.memset(w2t[:].bitcast(f32), 0.0).then_inc(s_ms, 1)
            vector.wait_ge(s_cw, 16)
            vector.wait_ge(s_ms, 1)
            cmp3 = ctw[:, 0:512].rearrange("p (k o) -> p k o", o=32)
            # two waves: the kj 0-3 columns first, so group 0's first
            # matmuls (gated on wave A + the small leading x chunk) start
            # as early as possible
            for j in range(4):
                vector.tensor_scalar(
                    w2v[j * 32:j * 32 + 32, 0:4, j * 32:j * 32 + 32],
                    cmp3[j * 32:j * 32 + 32, 0:4, :],
                    0.0, None, mybir.AluOpType.add).then_inc(s_w2, 1)
            for j in range(4):
                vector.tensor_scalar(
                    w2v[j * 32:j * 32 + 32, 4:K, j * 32:j * 32 + 32],
                    cmp3[j * 32:j * 32 + 32, 4:K, :],
                    0.0, None, mybir.AluOpType.add).then_inc(s_w2b, 1)
            vector.wait_ge(s_cstv, 16)
            vector.wait_ge(s_pad, 2)
            for g in range(GROUPS + 1):
                vector.wait_ge(s_pe, g + 1)
                gi = GROUPS if g == 1 else (0 if g == 0 else g - 1)  # g0, halo, g1..
                pg = K if g == 1 else 128
                src = halop[:] if g == 1 else yp[0:pg, ((gi % 2) * 512):((gi % 2) * 512) + WK]
                if g == 1:
                    vector.tensor_scalar(
                        yv4[0:pg, gi, :, 0:128],
                        src.rearrange("p (t s) -> p t s", t=2),
                        dvt[0:pg, :], mkt[:, :],
                        mybir.AluOpType.add,
                        mybir.AluOpType.mult).then_inc(s_evac, 1)
                else:
                    vector.tensor_scalar(
                        yv4[0:pg, gi, :, 0:128],
                        src.rearrange("p (t s) -> p t s", t=2),
                        dvt[0:pg, :], None,
                        mybir.AluOpType.add).then_inc(s_evac, 1)
                if gi == 5:
                    vector.wait_ge(s_conv, 1)
                    vector.tensor_scalar(
                        out_sb[0:64, :], convp[0:64, 0:512],
                        svt[0:64, :], bvt[0:64, :],
                        mybir.AluOpType.mult,
                        mybir.AluOpType.add).then_inc(s_out, 1)
            vector.wait_ge(s_conv, 3)
            vector.tensor_scalar(
                out_sb[64:128, :], convp[64:128, 512:1024],
                svt[64:128, :], bvt[64:128, :],
                mybir.AluOpType.mult,
                mybir.AluOpType.add).then_inc(s_out, 1)

        @block.scalar
        def _(scalar: bass.BassEngine):
            scalar.wait_ge(s_out, 1)
            scalar.dma_start(out=out_d[0:64, :],
                             in_=out_sb[0:64, :]).then_inc(s_od, 16)
            scalar.wait_ge(s_out, 2)
            scalar.dma_start(out=out_d[64:128, :],
                             in_=out_sb[64:128, :]).then_inc(s_od, 16)
            # the [64, x] stores only have data descriptors on half the SDMA
            # engines, so their own sems can fire before the bytes land; a
            # trailing full-width guard DMA on the same ring (per-engine
            # FIFO) closes the race.
            scalar.dma_start(out=guard[:], in_=xin[0:128, 0:8]).then_inc(s_og, 16)
            scalar.wait_ge(s_og, 16)

    nc.finalize()
    return nc


def prepare_in_maps(x, Wr, br, Wi, bi, dw_kernel, dw_bias, gamma, beta,
                    moving_mean, moving_var):
    x = np.ascontiguousarray(np.asarray(x, np.float32))[..., 0]  # (2, 4096, 4096)
    M3, Lmain, Lhalo, dvec, scalev, bvec = _build_consts(
        np.asarray(Wr), np.asarray(br), np.asarray(Wi), np.asarray(bi),
        np.asarray(dw_kernel), np.asarray(dw_bias), np.asarray(gamma),
        np.asarray(beta), np.asarray(moving_mean), np.asarray(moving_var))
    cst = np.zeros((128, CST16_W), np.float16)
    # 32-diag compact stage-1 data: partition p = hi*16+ki, free (kj,
    # c 0..32); the 16x16 M3 block sits at c = 16*((p//16) % 2) so one
    # 32-aligned copy per 32-partition group expands it on-chip into the
    # per-kj [128,128] block-diag stationaries.  x is pre-scaled by 2 on
    # the host so N(0,1) data sits in e3m4's normal range (|v| >= 0.25
    # <=> |x| >= 0.125); the 1/2 folds in here.
    cmp2 = np.zeros((128, K, 32), np.float32)
    for hi in range(8):
        off = 16 * (hi % 2)
        cmp2[hi * 16:hi * 16 + 16, :, off:off + 16] = 0.5 * M3
    cstw = np.zeros((128, 544), np.float16)
    cstw[:, 0:512] = cmp2.reshape(128, 512).astype(np.float16)
    # transposed-halo stationaries: partition p = (kj - 8b)*16 + ki,
    # cols 512 + b*16 + oc  <-  0.5 * M3[ki, kj, oc]
    wh = (0.5 * M3).transpose(1, 0, 2)            # [kj, ki, oc]
    for b in range(2):
        cstw[:, 512 + b * 16:512 + b * 16 + 16] = \
            wh[b * 8:(b + 1) * 8].reshape(128, D).astype(np.float16)
    cst[:, COL_LM:COL_LM + 192] = \
        Lmain.transpose(1, 0, 2).reshape(128, 3 * 64).astype(np.float16)
    cst[:, COL_LH:COL_LH + 192] = \
        Lhalo.transpose(1, 0, 2).reshape(128, 3 * 64).astype(np.float16)
    cv = np.zeros((128, CSTV_W), np.float32)
    cv[:, CV_DV] = dvec
    cv[:, CV_SV] = scalev
    cv[:, CV_BV] = bvec

    in_maps = []
    for core in range(N_CORES):
        b, quarter = core // 4, core % 4
        r0 = quarter * GROUPS * GROUP_PIX
        slab = np.zeros((SLAB_ROWS, W), ml_dtypes.float8_e3m4)
        # column order (kj, wi-parity, s): contiguous stage-1 rhs slices
        # and parity-split PSUM for contiguous stage-2 conv taps
        slab[:] = (2.0 * x[b, r0:r0 + SLAB_ROWS]).astype(
            ml_dtypes.float8_e3m4)[:, _PERM]
        # halo patch row, transposed: [16, 4096] (kj,par,s)-ordered ->
        # [128 (kj-half,ki), 2*256]
        xh = np.zeros((128, 2 * WK), ml_dtypes.float8_e3m4)
        if r0 + SLAB_ROWS < H:
            xhal = (2.0 * x[b, r0 + SLAB_ROWS:r0 + SLAB_ROWS + K]).astype(
                ml_dtypes.float8_e3m4)[:, _PERM]
            t = xhal.reshape(K, K, WK).transpose(1, 0, 2)     # [kj, ki, c]
            xh = np.ascontiguousarray(np.concatenate(
                [t[0:8].reshape(128, WK), t[8:16].reshape(128, WK)],
                axis=1))
        cvc = cv.copy()
        cvc[0:16, CV_MK] = 0.0 if quarter == 3 else 1.0
        in_maps.append({"xin": slab, "cst": cst, "cstv": cvc,
                        "cstw": cstw, "xinh": xh})
    return in_maps


def gather(results):
    out = np.zeros((B, 128, 128, D), np.float32)
    for core in range(N_CORES):
        arr = np.asarray(results[core]["out"])         # [128, 512]
        # partition = (half, r, oc), col = (g', s); block row = (h*4+g')*4+r
        arr = arr.reshape(2, OUT_R, D, 4, 128).transpose(0, 3, 1, 4, 2)
        arr = arr.reshape(32, 128, D)
        b, quarter = core // 4, core % 4
        out[b, quarter * 32:quarter * 32 + 32] = arr
    return out


_NC_CACHE = None


def _ensure_ntff_hook():
    """The agent image's `antenv` lacks `axon_hooks`; bass_utils imports it
    unconditionally when trace=True. Shim the module and register the
    ctypes-based NTFF hook from trn_agent_boot if available."""
    try:
        import antenv.axon_hooks  # noqa: F401
        return True
    except ImportError:
        pass
    try:
        import types
        import antenv
        from trn_agent_boot.trn_boot import _ntff_profile_via_ctypes

        mod = types.ModuleType("antenv.axon_hooks")
        state = {"hook": None}
        mod.set_axon_ntff_profile_hook = lambda h: state.__setitem__("hook", h)
        mod.get_axon_ntff_profile_hook = lambda: state["hook"]
        sys.modules["antenv.axon_hooks"] = mod
        antenv.axon_hooks = mod
        so_path = "/opt/axon/libaxon_pjrt.so"
        if os.path.exists(so_path):
            mod.set_axon_ntff_profile_hook(_ntff_profile_via_ctypes(so_path))
        return True
    except Exception:
        return False


def kernel(x, Wr, br, Wi, bi, dw_kernel, dw_bias, gamma, beta,
           moving_mean, moving_var, _trace=None):
    global LAST_RESULT, _NC_CACHE
    from concourse.bass_utils import run_bass_kernel_spmd

    in_maps = prepare_in_maps(x, Wr, br, Wi, bi, dw_kernel, dw_bias, gamma,
                              beta, moving_mean, moving_var)
    if _NC_CACHE is None:
        _NC_CACHE = _build_nc()
    nc = _NC_CACHE

    trace = (os.environ.get("BASS_TRACE", "") not in ("", "0")
             if _trace is None else _trace)
    if trace and not _ensure_ntff_hook():
        trace = False
    res = run_bass_kernel_spmd(nc, in_maps, list(range(N_CORES)), trace=trace)
    LAST_RESULT = res
    return gather(res.results)


if __name__ == "__main__":
    rng = np.random.default_rng(0)
    inputs = {
        "x": rng.standard_normal((B, H, W, 1), np.float32),
        "Wr": rng.standard_normal((256, D), np.float32) / 16,
        "br": rng.standard_normal(D).astype(np.float32) * 0.02,
        "Wi": rng.standard_normal((256, D), np.float32) / 16,
        "bi": rng.standard_normal(D).astype(np.float32) * 0.02,
        "dw_kernel": rng.standard_normal((3, 3, D, 1), np.float32) * 0.1,
        "dw_bias": rng.standard_normal(D).astype(np.float32) * 0.02,
        "gamma": 1 + 0.1 * rng.standard_normal(D).astype(np.float32),
        "beta": 0.1 * rng.standard_normal(D).astype(np.float32),
        "moving_mean": 0.1 * rng.standard_normal(D).astype(np.float32),
        "moving_var": rng.uniform(0.5, 1.5, D).astype(np.float32),
    }
    out = kernel(**inputs)
    print("out", out.shape, out.dtype, float(np.abs(out).max()))
